# revision 1
# baseline (speedup 1.0000x reference)
"""CLIP encoder layer (B=8, S=1024, D=1024, H=16, FF=4096, fp32) on 8 TRN2
NeuronCores, data-parallel over batch (one batch element per core).

Strategy
--------
- Host pre-transposes all weights (x @ W.T needs W with the contraction dim on
  SBUF partitions) and folds the 1/sqrt(Dh) query scale into Wq/bq (exact:
  0.125 is a power of two).
- Activations are kept feature-major on chip (xT = x.T per core), so no
  on-device transposes are needed anywhere; the host transposes the final
  output back.
- All matmuls run in bf16 with fp32 PSUM accumulation (1 PE cycle/row).
  LayerNorm statistics are partition-dim reductions done as ones-vector
  matmuls; the residual stream stays fp32.
- Softmax: scores are O(1) here so exp() without max-subtraction is safe;
  the denominator is obtained for free as a 65th lhsT column of ones in the
  probs @ V matmul, and applied after PV via a K=1 broadcast matmul.
"""

import sys

import numpy as np

if "/opt/trn_rl_repo" not in sys.path:
    sys.path.insert(0, "/opt/trn_rl_repo")

import ml_dtypes

B, S, D = 8, 1024, 1024
H, Dh = 16, 64
FF = 4096
EPS = 1e-5
P = 128
DC = D // P          # 8 feature chunks
SC = S // P          # 8 token chunks
FC = FF // P         # 32 ff chunks
HALF = S // 2        # 512
N_CORES = 8

_BUILD_CACHE = {}


# ---------------------------------------------------------------------------
# Workaround: this walrus build rejects instructions carrying more than one
# sync-wait command. Split excess on_wait entries onto NoOp instructions
# inserted before the offending instruction on the same engine.
# ---------------------------------------------------------------------------
def _patch_wait_split():
    import json

    import concourse.bass as bass

    if getattr(bass.Bass, "_wait_split_patched", False):
        return
    bass.Bass._wait_split_patched = True
    MAX_WAITS = 1
    orig = bass.Bass.to_json_bytes

    def _split(bir: bytes) -> bytes:
        m = json.loads(bir)
        n = 0
        changed = False
        for fn in m.get("functions", []):
            for bb in fn.get("blocks", []):
                # Dedup consecutive identical PE weight loads: the PE array
                # keeps the stationary operand across Matmults, so a reload
                # of the same AP is pure overhead. Convert to NoOp (keeps
                # sync_info and ordering).
                last_ldw = None
                for inst in bb.get("instructions", []):
                    if inst.get("engine") != "PE":
                        continue
                    op = inst.get("opcode")
                    if op == "Ldweights":
                        sig = json.dumps(inst.get("ins"), sort_keys=True)
                        if sig == last_ldw:
                            inst["opcode"] = "NoOp"
                            inst["ins"] = []
                            inst["outs"] = []
                            changed = True
                        else:
                            last_ldw = sig
                    elif op not in ("Matmult", "NoOp"):
                        last_ldw = None
                out = []
                for inst in bb.get("instructions", []):
                    si = inst.get("sync_info")
                    waits = (si or {}).get("on_wait") or []
                    if len(waits) > MAX_WAITS:
                        changed = True
                        extra, keep = waits[:-MAX_WAITS], waits[-MAX_WAITS:]
                        for i in range(0, len(extra), MAX_WAITS):
                            n += 1
                            out.append({
                                "debug": inst.get("debug", 0),
                                "engine": inst["engine"],
                                "ins": [],
                                "outs": [],
                                "name": f"I-wsplit-{n}",
                                "opcode": "NoOp",
                                "sync_info": {
                                    "on_update": [],
                                    "on_wait": extra[i:i + MAX_WAITS],
                                },
                            })
                        si["on_wait"] = keep
                    out.append(inst)
                bb["instructions"] = out
        return json.dumps(m).encode() if changed else bir

    def to_json_bytes(self):
        return _split(orig(self))

    bass.Bass.to_json_bytes = to_json_bytes


def _build(reps: int, triv: dict):
    """Build the Bass program. `triv[name]` flags all-zero biases / trivial LN
    affines so the specialized ops can be skipped (inputs are fixed at build
    time; generic paths are emitted when the flag is False)."""
    import concourse.bass as bass
    import concourse.mybir as mybir
    from concourse.tile import TileContext

    _patch_wait_split()

    f32 = mybir.dt.float32
    bf16 = mybir.dt.bfloat16
    AF = mybir.ActivationFunctionType
    OP = mybir.AluOpType

    nc = bass.Bass()

    # ---- DRAM I/O ----
    xT_d = nc.dram_tensor("xT", [D, S], f32, kind="ExternalInput")
    wq_d = nc.dram_tensor("wq", [D, D], bf16, kind="ExternalInput")
    wk_d = nc.dram_tensor("wk", [D, D], bf16, kind="ExternalInput")
    wv_d = nc.dram_tensor("wv", [D, D], bf16, kind="ExternalInput")
    wo_d = nc.dram_tensor("wo", [D, D], bf16, kind="ExternalInput")
    w1_d = nc.dram_tensor("w1", [D, FF], bf16, kind="ExternalInput")
    w2_d = nc.dram_tensor("w2", [FF, D], bf16, kind="ExternalInput")
    bias_d = {}
    for name, n in [("bq", D), ("bk", D), ("bv", D), ("bo", D),
                    ("b1", FF), ("b2", D),
                    ("ln1_g", D), ("ln1_b", D), ("ln2_g", D), ("ln2_b", D)]:
        if not triv[name]:
            bias_d[name] = nc.dram_tensor(name, [1, n], f32, kind="ExternalInput")
    outT_d = nc.dram_tensor("outT", [D, S], f32, kind="ExternalOutput")
    # scratch for the [1,S] -> [128,S] partition broadcasts (per-LN a/c rows)
    a_scr = nc.dram_tensor("a_scr", [1, S], f32, kind="Internal")
    c_scr = nc.dram_tensor("c_scr", [1, S], f32, kind="Internal")
    a2_scr = nc.dram_tensor("a2_scr", [1, S], f32, kind="Internal")
    c2_scr = nc.dram_tensor("c2_scr", [1, S], f32, kind="Internal")

    with TileContext(nc) as tc:
        _emit(nc, tc, mybir, f32, bf16, AF, OP, reps, triv,
              xT_d, wq_d, wk_d, wv_d, wo_d, w1_d, w2_d, bias_d, outT_d,
              a_scr, c_scr, a2_scr, c2_scr)
    return nc


def _emit(nc, tc, mybir, f32, bf16, AF, OP, reps, triv,
          xT_d, wq_d, wk_d, wv_d, wo_d, w1_d, w2_d, bias_d, outT_d,
          a_scr, c_scr, a2_scr, c2_scr):
    from contextlib import ExitStack

    ctx = ExitStack()
    with ctx:
        root = ctx.enter_context(tc.tile_pool(name="root", bufs=1))
        psum = ctx.enter_context(tc.tile_pool(name="ps", bufs=1, space="PSUM"))

        # constants
        ones_col = root.tile([P, 1], bf16, name="ones_col")      # LN stats lhsT
        nc.vector.memset(ones_col, 1.0)
        ones65 = root.tile([65, 64], bf16, name="ones65")        # denom bcast lhsT
        nc.vector.memset(ones65, 1.0)
        eps_t = root.tile([1, 1], f32, name="eps_t")
        nc.vector.memset(eps_t, EPS)

        # striped per-feature vectors [128, n/128] (only when non-trivial)
        bias_sb = {}
        for name, n in [("bq", D), ("bk", D), ("bo", D), ("b1", FF), ("b2", D),
                        ("ln1_g", D), ("ln1_b", D), ("ln2_g", D), ("ln2_b", D)]:
            if not triv[name]:
                t = root.tile([P, n // P], f32, name=f"sb_{name}")
                nc.sync.dma_start(out=t, in_=bias_d[name][:, :].rearrange(
                    "o (c p) -> (o p) c", p=P))
                bias_sb[name] = t
        bv_b = None
        if not triv["bv"]:
            bv_b = root.tile([P, D], f32, name="bv_b")
            nc.sync.dma_start(out=bv_b, in_=bias_d["bv"][:, :].to_broadcast([P, D]))

        # -------- LayerNorm, feature-major (stats via ones-matmuls) --------
        def layer_norm(src_of, dst_bf16, g_name, b_name, ascr, cscr, pool_name):
            ln = ExitStack()
            with ln:
                pl = ln.enter_context(tc.tile_pool(name=pool_name, bufs=1))
                def src_f32(dc, cols):
                    return src_of(pl, dc, cols)
                a_vec = pl.tile([1, S], f32, name=f"{pool_name}_a")
                c_vec = pl.tile([1, S], f32, name=f"{pool_name}_c")
                a_b = pl.tile([P, S], f32, name=f"{pool_name}_ab")
                c_b = pl.tile([P, S], f32, name=f"{pool_name}_cb")
                for hf in range(2):
                    cols = slice(hf * HALF, (hf + 1) * HALF)
                    st = psum.tile([1, HALF], f32, tag="st", bufs=2, name="st_sum")
                    st2 = psum.tile([1, HALF], f32, tag="st", bufs=2, name="st_sq")
                    for dc in range(DC):
                        xc = pl.tile([P, HALF], bf16, tag="lnxc", bufs=2,
                                     name="ln_xc")
                        nc.scalar.activation(out=xc, in_=src_f32(dc, cols),
                                             func=AF.Copy)
                        nc.tensor.matmul(st[0:1, :], ones_col, xc,
                                         start=(dc == 0), stop=(dc == DC - 1))
                        sq = pl.tile([P, HALF], bf16, tag="lnsq", bufs=2,
                                     name="ln_sq")
                        nc.scalar.activation(out=sq, in_=src_f32(dc, cols),
                                             func=AF.Square)
                        nc.tensor.matmul(st2[0:1, :], ones_col, sq,
                                         start=(dc == 0), stop=(dc == DC - 1))
                    mu = pl.tile([1, HALF], f32, tag="lnmu", bufs=1, name="ln_mu")
                    nc.vector.tensor_scalar_mul(mu, st[0:1, :], 1.0 / D)
                    msq = pl.tile([1, HALF], f32, tag="lnmsq", bufs=1,
                                  name="ln_msq")
                    nc.vector.tensor_scalar_mul(msq, st2[0:1, :], 1.0 / D)
                    musq = pl.tile([1, HALF], f32, tag="lnmusq", bufs=1,
                                   name="ln_musq")
                    nc.vector.tensor_tensor(out=musq, in0=mu, in1=mu, op=OP.mult)
                    # var (in place over msq), sd (in place over musq)
                    nc.vector.tensor_tensor(out=msq, in0=msq, in1=musq,
                                            op=OP.subtract)
                    nc.scalar.activation(out=musq, in_=msq, func=AF.Sqrt,
                                         bias=eps_t)
                    nc.vector.reciprocal(out=a_vec[0:1, cols], in_=musq)
                    nc.vector.tensor_tensor(out=musq, in0=mu,
                                            in1=a_vec[0:1, cols], op=OP.mult)
                    nc.vector.tensor_scalar_mul(c_vec[0:1, cols], musq, -1.0)
                    # per-half broadcast via DRAM round-trip
                    nc.sync.dma_start(out=ascr[:, cols], in_=a_vec[0:1, cols])
                    nc.sync.dma_start(out=cscr[:, cols], in_=c_vec[0:1, cols])
                    nc.sync.dma_start(
                        out=a_b[:, cols],
                        in_=ascr[0:1, cols].to_broadcast([P, HALF]))
                    nc.sync.dma_start(
                        out=c_b[:, cols],
                        in_=cscr[0:1, cols].to_broadcast([P, HALF]))
                trivial_affine = triv[g_name] and triv[b_name]
                for dc in range(DC):
                    for hf in range(2):
                        cols = slice(hf * HALF, (hf + 1) * HALF)
                        tn = pl.tile([P, HALF], f32, tag="lntn", bufs=2,
                                     name="ln_tn")
                        nc.vector.tensor_tensor(out=tn, in0=src_f32(dc, cols),
                                                in1=a_b[:, cols], op=OP.mult)
                        if trivial_affine:
                            nc.vector.tensor_tensor(out=dst_bf16[:, dc, cols],
                                                    in0=tn, in1=c_b[:, cols],
                                                    op=OP.add)
                        else:
                            nc.vector.tensor_tensor(out=tn, in0=tn,
                                                    in1=c_b[:, cols], op=OP.add)
                            nc.vector.tensor_scalar(
                                dst_bf16[:, dc, cols], tn,
                                bias_sb[g_name][:, dc:dc + 1],
                                bias_sb[b_name][:, dc:dc + 1],
                                OP.mult, OP.add)

        def body():
            bctx = ExitStack()
            with bctx:
                # x2T outlives the Wo-phase pools -> opened first
                p_x2 = bctx.enter_context(tc.tile_pool(name="p_x2", bufs=1))

                woctx = ExitStack()
                p_w = woctx.enter_context(tc.tile_pool(name="p_w", bufs=1))
                p_at = woctx.enter_context(tc.tile_pool(name="p_at", bufs=1))
                attnTb = p_at.tile([P, DC, S], bf16, name="attnTb")
                p_asm = woctx.enter_context(tc.tile_pool(name="p_asm", bufs=1))

                actx = ExitStack()
                p_qk = actx.enter_context(tc.tile_pool(name="p_qk", bufs=1))
                qTb = p_qk.tile([P, DC, S], bf16, name="qTb")
                kTb = p_qk.tile([P, DC, S], bf16, name="kTb")
                p_v = actx.enter_context(tc.tile_pool(name="p_v", bufs=1))
                vb = p_v.tile([P, SC, H, 65], bf16, name="vb")
                nc.vector.memset(vb[:, :, :, 64:65], 1.0)

                # ---------------- LN1 -> hTb ----------------
                hctx = ExitStack()
                p_h = hctx.enter_context(tc.tile_pool(name="p_h", bufs=1))
                hTb = p_h.tile([P, DC, S], bf16, name="hTb")
                def x_chunk(pl, dc, cols):
                    t = pl.tile([P, cols.stop - cols.start], f32, tag="xsrc",
                                bufs=2, name="x_chunk")
                    nc.sync.dma_start(
                        out=t, in_=xT_d[dc * P:(dc + 1) * P, cols])
                    return t
                layer_norm(x_chunk, hTb, "ln1_g", "ln1_b", a_scr, c_scr, "ln1")

                # ---------------- QKV ----------------
                for wname, w_dram, dst, bias in [
                        ("wq", wq_d, qTb, "bq"), ("wk", wk_d, kTb, "bk")]:
                    wt = p_w.tile([P, DC, D], bf16, tag="wqkv", bufs=2,
                                  name=f"t_{wname}")
                    nc.sync.dma_start(out=wt, in_=w_dram[:, :].rearrange(
                        "(c p) i -> p c i", p=P))
                    for ic in range(DC):
                        pss = [psum.tile([P, HALF], f32, tag="mm", bufs=3,
                                         name=f"ps_{wname}{hf}")
                               for hf in range(2)]
                        for dc in range(DC):
                            for hf in range(2):
                                cols = slice(hf * HALF, (hf + 1) * HALF)
                                nc.tensor.matmul(
                                    pss[hf], wt[:, dc, ic * P:(ic + 1) * P],
                                    hTb[:, dc, cols],
                                    start=(dc == 0), stop=(dc == DC - 1))
                        for hf in range(2):
                            cols = slice(hf * HALF, (hf + 1) * HALF)
                            if triv[bias]:
                                nc.scalar.activation(out=dst[:, ic, cols],
                                                     in_=pss[hf], func=AF.Copy)
                            else:
                                nc.scalar.activation(
                                    out=dst[:, ic, cols], in_=pss[hf],
                                    func=AF.Copy,
                                    bias=bias_sb[bias][:, ic:ic + 1])

                # v (token-major, heads strided by 65 with a ones column)
                wvt = p_w.tile([P, DC, D], bf16, tag="wqkv", bufs=2, name="t_wv")
                nc.sync.dma_start(out=wvt, in_=wv_d[:, :].rearrange(
                    "(c p) i -> p c i", p=P))
                for sc in range(SC):
                    pss = [psum.tile([P, HALF], f32, tag="mm", bufs=3,
                                     name=f"ps_v{hf}") for hf in range(2)]
                    for dc in range(DC):
                        for hf in range(2):
                            cols = slice(hf * HALF, (hf + 1) * HALF)
                            nc.tensor.matmul(
                                pss[hf], hTb[:, dc, sc * P:(sc + 1) * P],
                                wvt[:, dc, cols],
                                start=(dc == 0), stop=(dc == DC - 1))
                    for hf in range(2):
                        cols = slice(hf * HALF, (hf + 1) * HALF)
                        h0 = hf * 8
                        src = pss[hf][:, :].rearrange("p (h c) -> p h c", h=8)
                        dst = vb[:, sc, h0:h0 + 8, 0:64]
                        if triv["bv"]:
                            nc.vector.tensor_copy(out=dst, in_=src)
                        else:
                            bvv = bv_b[:, cols].rearrange("p (h c) -> p h c", h=8)
                            nc.vector.tensor_tensor(out=dst, in0=src, in1=bvv,
                                                    op=OP.add)

                # ---------------- attention ----------------
                hctx.close()  # hTb dead
                p_probs = actx.enter_context(tc.tile_pool(name="p_probs", bufs=1))

                for h in range(H):
                    po = (h % 2) * 64          # partition offset of this head
                    hc = h // 2                # chunk of qTb/kTb
                    probs2 = [p_probs.tile([P, SC, HALF], bf16, tag="probs",
                                           bufs=3, name=f"probs{hf}")
                              for hf in range(2)]
                    for skc in range(SC):
                        pss = [psum.tile([P, HALF], f32, tag="mm", bufs=3,
                                         name=f"ps_scores{hf}")
                               for hf in range(2)]
                        for hf in range(2):
                            cols = slice(hf * HALF, (hf + 1) * HALF)
                            nc.tensor.matmul(
                                pss[hf],
                                kTb[po:po + 64, hc, skc * P:(skc + 1) * P],
                                qTb[po:po + 64, hc, cols],
                                start=True, stop=True)
                        for hf in range(2):
                            nc.scalar.activation(out=probs2[hf][:, skc, :],
                                                 in_=pss[hf], func=AF.Exp)
                    # PV with fused denominator (ones column -> row 64)
                    ps_pv2 = [psum.tile([65, HALF], f32, tag="pv", bufs=2,
                                        name=f"ps_pv{hf}") for hf in range(2)]
                    for skc in range(SC):
                        for hf in range(2):
                            nc.tensor.matmul(
                                ps_pv2[hf][0:65, :], vb[:, skc, h, :],
                                probs2[hf][:, skc, :],
                                start=(skc == 0), stop=(skc == SC - 1))
                    for hf in range(2):
                        cols = slice(hf * HALF, (hf + 1) * HALF)
                        ps_pv = ps_pv2[hf]
                        rec = p_asm.tile([65, HALF], bf16, tag="rec", bufs=3,
                                         name="rec")
                        with nc.allow_low_precision(reason="softmax denom bcast"):
                            nc.vector.reciprocal(out=rec[64:65, :],
                                                 in_=ps_pv[64:65, :])
                        ps_r = psum.tile([64, HALF], f32, tag="rec", bufs=1,
                                         name="ps_rec")
                        nc.tensor.matmul(ps_r[0:64, :], ones65[64:65, :],
                                         rec[64:65, :], start=True, stop=True)
                        rb = p_asm.tile([64, HALF], f32, tag="rb", bufs=3,
                                        name="rb")
                        nc.scalar.activation(out=rb, in_=ps_r[0:64, :], func=AF.Copy)
                        if po == 0:
                            nc.vector.tensor_tensor(
                                out=attnTb[0:64, hc, cols], in0=ps_pv[0:64, :],
                                in1=rb, op=OP.mult)
                        else:
                            tmp = p_asm.tile([64, HALF], bf16, tag="tmp", bufs=3,
                                             name="attn_tmp")
                            nc.vector.tensor_tensor(out=tmp, in0=ps_pv[0:64, :],
                                                    in1=rb, op=OP.mult)
                            nc.sync.dma_start(out=attnTb[64:128, hc, cols], in_=tmp)

                # ---------------- Wo + residual -> x2T ----------------
                actx.close()  # qTb/kTb/vb/probs dead
                x2T = p_x2.tile([P, DC, S], f32, name="x2T")
                wot = p_w.tile([P, DC, D], bf16, tag="wqkv", bufs=2, name="t_wo")
                nc.sync.dma_start(out=wot, in_=wo_d[:, :].rearrange(
                    "(c p) i -> p c i", p=P))
                for ic in range(DC):
                    xres = p_asm.tile([P, S], f32, tag="xres", bufs=2,
                                      name="xres")
                    nc.sync.dma_start(out=xres,
                                      in_=xT_d[ic * P:(ic + 1) * P, :])
                    pss = [psum.tile([P, HALF], f32, tag="mm", bufs=3,
                                     name=f"ps_wo{hf}") for hf in range(2)]
                    for dc in range(DC):
                        for hf in range(2):
                            cols = slice(hf * HALF, (hf + 1) * HALF)
                            nc.tensor.matmul(
                                pss[hf], wot[:, dc, ic * P:(ic + 1) * P],
                                attnTb[:, dc, cols],
                                start=(dc == 0), stop=(dc == DC - 1))
                    for hf in range(2):
                        cols = slice(hf * HALF, (hf + 1) * HALF)
                        if triv["bo"]:
                            nc.vector.tensor_tensor(out=x2T[:, ic, cols],
                                                    in0=pss[hf],
                                                    in1=xres[:, cols], op=OP.add)
                        else:
                            t = p_asm.tile([P, HALF], f32, tag="wo_t", bufs=2,
                                           name="wo_t")
                            nc.scalar.activation(out=t, in_=pss[hf], func=AF.Copy,
                                                 bias=bias_sb["bo"][:, ic:ic + 1])
                            nc.vector.tensor_tensor(out=x2T[:, ic, cols], in0=t,
                                                    in1=xres[:, cols], op=OP.add)

                # ---------------- LN2 / MLP ----------------
                woctx.close()  # xT, weights pool, attnTb, asm dead
                p_mlp = bctx.enter_context(tc.tile_pool(name="p_mlp", bufs=1))
                g1Tb = p_mlp.tile([P, FC, S], bf16, name="g1Tb")
                h2ctx = ExitStack()
                p_h2 = h2ctx.enter_context(tc.tile_pool(name="p_h2", bufs=1))
                h2Tb = p_h2.tile([P, DC, S], bf16, name="h2Tb")
                layer_norm(lambda pl, dc, cols: x2T[:, dc, cols], h2Tb,
                           "ln2_g", "ln2_b", a2_scr, c2_scr, "ln2")

                # MLP1 + quick-gelu -> g1Tb
                w1ctx = ExitStack()
                p_w1 = w1ctx.enter_context(tc.tile_pool(name="p_w1", bufs=1))
                for ffb in range(8):
                    w1t = p_w1.tile([P, DC, 512], bf16, tag="w1t", bufs=2,
                                    name="w1t")
                    nc.sync.dma_start(
                        out=w1t,
                        in_=w1_d[:, ffb * 512:(ffb + 1) * 512].rearrange(
                            "(c p) i -> p c i", p=P))
                    for fq in range(4):
                        ffc = ffb * 4 + fq
                        pss = [psum.tile([P, HALF], f32, tag="mm", bufs=3,
                                         name=f"ps_m1{hf}") for hf in range(2)]
                        for dc in range(DC):
                            for hf in range(2):
                                cols = slice(hf * HALF, (hf + 1) * HALF)
                                nc.tensor.matmul(
                                    pss[hf], w1t[:, dc, fq * P:(fq + 1) * P],
                                    h2Tb[:, dc, cols],
                                    start=(dc == 0), stop=(dc == DC - 1))
                        for hf in range(2):
                            cols = slice(hf * HALF, (hf + 1) * HALF)
                            ps = pss[hf]
                            sig = p_mlp.tile([P, HALF], f32, tag="sig", bufs=2,
                                             name="sig")
                            if triv["b1"]:
                                nc.scalar.activation(out=sig, in_=ps,
                                                     func=AF.Sigmoid, scale=1.702)
                                nc.vector.tensor_tensor(
                                    out=g1Tb[:, ffc, cols], in0=ps, in1=sig,
                                    op=OP.mult)
                            else:
                                h1 = p_mlp.tile([P, HALF], f32, tag="h1", bufs=2,
                                                name="h1")
                                nc.vector.tensor_scalar(
                                    h1, ps, bias_sb["b1"][:, ffc:ffc + 1], None,
                                    OP.add)
                                nc.scalar.activation(out=sig, in_=h1,
                                                     func=AF.Sigmoid, scale=1.702)
                                nc.vector.tensor_tensor(
                                    out=g1Tb[:, ffc, cols], in0=h1, in1=sig,
                                    op=OP.mult)

                # MLP2 + residual -> outT (SBUF accumulation across 8 groups)
                w1ctx.close()  # w1 stream dead
                h2ctx.close()  # h2Tb dead
                out_acc = p_mlp.tile([P, DC, S], f32, name="out_acc")
                for grp in range(8):
                    w2t = p_mlp.tile([P, 4, D], bf16, tag="w2t", bufs=2,
                                     name="w2t")
                    nc.sync.dma_start(
                        out=w2t,
                        in_=w2_d[grp * 512:(grp + 1) * 512, :].rearrange(
                            "(c p) i -> p c i", p=P))
                    for ic in range(DC):
                        pss = [psum.tile([P, HALF], f32, tag="mm", bufs=3,
                                         name=f"ps_m2{hf}") for hf in range(2)]
                        for gq in range(4):
                            ffc = grp * 4 + gq
                            for hf in range(2):
                                cols = slice(hf * HALF, (hf + 1) * HALF)
                                nc.tensor.matmul(
                                    pss[hf], w2t[:, gq, ic * P:(ic + 1) * P],
                                    g1Tb[:, ffc, cols],
                                    start=(gq == 0), stop=(gq == 3))
                        for hf in range(2):
                            cols = slice(hf * HALF, (hf + 1) * HALF)
                            ps = pss[hf]
                            if grp == 0:
                                if triv["b2"]:
                                    nc.vector.tensor_tensor(
                                        out=out_acc[:, ic, cols], in0=ps,
                                        in1=x2T[:, ic, cols], op=OP.add)
                                else:
                                    t = p_mlp.tile([P, HALF], f32, tag="m2_t",
                                                   bufs=2, name="m2_t")
                                    nc.scalar.activation(
                                        out=t, in_=ps, func=AF.Copy,
                                        bias=bias_sb["b2"][:, ic:ic + 1])
                                    nc.vector.tensor_tensor(
                                        out=out_acc[:, ic, cols], in0=t,
                                        in1=x2T[:, ic, cols], op=OP.add)
                            else:
                                nc.vector.tensor_tensor(
                                    out=out_acc[:, ic, cols], in0=ps,
                                    in1=out_acc[:, ic, cols], op=OP.add)
                for ic in range(DC):
                    nc.sync.dma_start(out=outT_d[ic * P:(ic + 1) * P, :],
                                      in_=out_acc[:, ic, :])

        if reps == 1:
            body()
        else:
            with tc.For_i(0, reps, 1):
                body()


def _prepare_inputs(inputs):
    x = np.asarray(inputs["hidden_states"], np.float32)
    scale = Dh ** (-0.5)

    def bf(a):
        return np.ascontiguousarray(a).astype(ml_dtypes.bfloat16)

    wq = bf((scale * np.asarray(inputs["Wq"], np.float32)).T)
    wk = bf(np.asarray(inputs["Wk"], np.float32).T)
    wv = bf(np.asarray(inputs["Wv"], np.float32).T)
    wo = bf(np.asarray(inputs["Wo"], np.float32).T)
    w1 = bf(np.asarray(inputs["W1"], np.float32).T)
    w2 = bf(np.asarray(inputs["W2"], np.float32).T)

    vecs = {
        "bq": scale * np.asarray(inputs["bq"], np.float32),
        "bk": np.asarray(inputs["bk"], np.float32),
        "bv": np.asarray(inputs["bv"], np.float32),
        "bo": np.asarray(inputs["bo"], np.float32),
        "b1": np.asarray(inputs["b1"], np.float32),
        "b2": np.asarray(inputs["b2"], np.float32),
        "ln1_b": np.asarray(inputs["ln1_b"], np.float32),
        "ln2_b": np.asarray(inputs["ln2_b"], np.float32),
    }
    gvecs = {
        "ln1_g": np.asarray(inputs["ln1_g"], np.float32),
        "ln2_g": np.asarray(inputs["ln2_g"], np.float32),
    }
    triv = {k: bool(np.all(v == 0.0)) for k, v in vecs.items()}
    triv.update({k: bool(np.all(v == 1.0)) for k, v in gvecs.items()})

    shared = {"wq": wq, "wk": wk, "wv": wv, "wo": wo, "w1": w1, "w2": w2}
    for k, v in {**vecs, **gvecs}.items():
        if not triv[k]:
            shared[k] = np.ascontiguousarray(v.reshape(1, -1))

    in_maps = []
    for c in range(N_CORES):
        m = dict(shared)
        m["xT"] = np.ascontiguousarray(x[c].T)
        in_maps.append(m)
    return in_maps, triv


def _get_nc(reps, triv):
    key = (reps, tuple(sorted(triv.items())))
    if key not in _BUILD_CACHE:
        _BUILD_CACHE[key] = _build(reps, triv)
    return _BUILD_CACHE[key]


def kernel(**inputs):
    from concourse.bass_utils import run_bass_kernel_spmd

    in_maps, triv = _prepare_inputs(inputs)
    nc = _get_nc(1, triv)
    res = run_bass_kernel_spmd(nc, in_maps, core_ids=list(range(N_CORES)))
    out = np.empty((B, S, D), np.float32)
    for c in range(N_CORES):
        out[c] = res.results[c]["outT"].T
    return out


# used by test.py for timing (repeat the layer in-kernel to amortize overhead)
def run_with_reps(inputs, reps):
    from concourse.bass_utils import run_bass_kernel_spmd

    in_maps, triv = _prepare_inputs(inputs)
    nc = _get_nc(reps, triv)
    res = run_bass_kernel_spmd(nc, in_maps, core_ids=list(range(N_CORES)))
    out = np.empty((B, S, D), np.float32)
    for c in range(N_CORES):
        out[c] = res.results[c]["outT"].T
    return out

